# revision 13
# baseline (speedup 1.0000x reference)
"""GQA attention block (B=2, S=2048, D=2048, H=32, KVH=8, HD=64, RoPE) on 8
Trainium2 NeuronCores.

Sharding: core = (batch, kv-head pair). Core c handles batch c//4 and kv heads
{2*(c%4), 2*(c%4)+1} (i.e. q heads 8*(c%4)..8*(c%4)+7). Each core runs the full
chain for its heads: q/k/v projections + RoPE, attention, and its row-slice of
the output projection; the host sums the 4 partial wo-outputs per batch.

Device-side layout (host-side transforms are free):
- x is passed transposed (xT [D, S]) so projections produce qT/kT/vT with the
  head dim on partitions.
- RoPE uses the "half layout": wq/wk rows permuted per head to
  [even dims, odd dims]; cos/sin tables precomputed host-side. The 1/sqrt(HD)
  score scale is folded into the Q tables' consumer (scale=0.125).
- Scores are computed transposed ([keys, queries]) so softmax exp is a pure
  elementwise ACT op and the PV matmul consumes probsT directly as the moving
  operand. exp skips max-subtraction (scores bounded ~|7|).
- The softmax normalizer z is row HD of the PV output: V' carries a mask
  column at index HD (ones * mask), so PV yields z for free.
- All matmuls float32r (full-rate fp32 streaming).

Schedule: the attention phase is ACT(exp)-bound (~1.04us per kc iteration vs
~0.65us of PE work). All remaining PE work (q projections for chunks 1-3, the
first 12 wo chunks) is sliced into single-matmul "filler" steps issued between
kc iterations so the exp stream never starves. Preamble (K/V/V'/Q-chunk0) is
x-DMA-bound; attention starts as soon as K is fully projected.
"""

import sys

import numpy as np

if "/opt/trn_rl_repo" not in sys.path:
    sys.path.insert(0, "/opt/trn_rl_repo")

B, S, D = 2, 2048, 2048
H, KVH = 32, 8
HD = D // H            # 64
NREP = H // KVH        # 4
ROPE_THETA = 10000.0
N_CORES = 8
P = 128
NQ = 512               # q rows per core (8 heads * 64)
NKV = 128              # k/v rows per core (2 kv heads * 64)
KO = D // P            # 16 contraction chunks for projections
SC = S // 512          # 4 column chunks of 512
KH = 2                 # x streams in [P, KH, 512] tiles
NKQ = KO // KH         # 8 quarter-tiles per column chunk


def _rope_tables():
    """cos/sin tables [P, S] matching the qT/kT partition layout.

    Partition layout per 64-row head block: rows 0:32 = "a" (even dims),
    rows 32:64 = "b" (odd dims). a' = a*cos - b*sin ; b' = a*sin + b*cos.
    The in0 of the fused swap-multiply reads the OTHER block, so the sin
    table carries -sin on a-rows and +sin on b-rows.
    """
    freqs = (1.0 / (ROPE_THETA **
                    (np.arange(0, HD, 2, dtype=np.float32) / np.float32(HD))))
    freqs = freqs.astype(np.float32)                                  # [32]
    ang = (np.arange(S, dtype=np.float32)[None, :] * freqs[:, None])  # [32, S]
    cos = np.cos(ang).astype(np.float32)
    sin = np.sin(ang).astype(np.float32)
    ctab = np.concatenate([cos, cos, cos, cos], axis=0)               # [128, S]
    stab = np.concatenate([-sin, sin, -sin, sin], axis=0)             # [128, S]
    return ctab, stab


def _build_bass(reps: int = 1):
    import concourse.bass as bass  # noqa: F401
    import concourse.tile as tile
    from concourse import bacc, mybir
    from concourse.masks import make_identity

    F32 = mybir.dt.float32
    F32R = mybir.dt.float32r
    EXP = mybir.ActivationFunctionType.Exp
    COPY = mybir.ActivationFunctionType.Copy
    MULT = mybir.AluOpType.mult
    ADD = mybir.AluOpType.add

    nc = bacc.Bacc("TRN2", target_bir_lowering=False, debug=False,
                   num_devices=N_CORES)

    xT = nc.dram_tensor("xT", [D, S], F32R, kind="ExternalInput")
    wqT = nc.dram_tensor("wqT", [D, NQ], F32R, kind="ExternalInput")
    wkT = nc.dram_tensor("wkT", [D, NKV], F32R, kind="ExternalInput")
    wvT = nc.dram_tensor("wvT", [D, NKV], F32R, kind="ExternalInput")
    woT = nc.dram_tensor("woT", [NQ, D], F32R, kind="ExternalInput")
    ck = nc.dram_tensor("ck", [P, S], F32, kind="ExternalInput")
    sk = nc.dram_tensor("sk", [P, S], F32, kind="ExternalInput")
    maskT = nc.dram_tensor("maskT", [P, KO], F32, kind="ExternalInput")
    part = nc.dram_tensor("part", [S, D], F32, kind="ExternalOutput")

    xT_r = xT.ap().rearrange("(ko p) s -> p ko s", p=P)     # [128, 16, 2048]
    wqT_r = wqT.ap().rearrange("(ko p) m -> p ko m", p=P)   # [128, 16, 512]
    wkT_r = wkT.ap().rearrange("(ko p) m -> p ko m", p=P)   # [128, 16, 128]
    wvT_r = wvT.ap().rearrange("(ko p) m -> p ko m", p=P)   # [128, 16, 128]
    woT_r = woT.ap().rearrange("(dk p) e -> p dk e", p=P)   # [128, 4, 2048]

    with tile.TileContext(nc) as tc:
      for rep in range(reps):
        with tc.tile_pool(name="persist", bufs=1) as persist, \
             tc.tile_pool(name="probs", bufs=3) as prpool, \
             tc.tile_pool(name="nrm", bufs=2) as nrmpool, \
             tc.tile_pool(name="sw", bufs=2) as swpool, \
             tc.tile_pool(name="mmPS", bufs=2, space="PSUM") as mmps, \
             tc.tile_pool(name="attnPS", bufs=2, space="PSUM") as spool, \
             tc.tile_pool(name="pvPS", bufs=2, space="PSUM") as pvpool:

            qsb = [persist.tile([P, S], F32R, tag=f"qsb{m}", name=f"qsb{m}_{rep}")
                   for m in range(4)]
            kab = persist.tile([P, S], F32R, tag="kab")
            # V' layout: col 0 = mask (softmax z source), cols 1:65 = V dims
            vpr = [persist.tile([P, KO, HD + 1], F32R, tag=f"vpr{i}",
                                name=f"vpr{i}_{rep}")
                   for i in range(2)]
            msk = persist.tile([P, KO], F32, tag="msk")
            ident = persist.tile([P, P], F32, tag="ident")
            tab_ck = persist.tile([P, S], F32, tag="tab_ck")
            tab_sk = persist.tile([P, S], F32, tag="tab_sk")

            nc.sync.dma_start(msk[:], maskT.ap())
            make_identity(nc, ident[:])

            def rope_evac(ps, dst_tile, s0, scale):
                """dst[:, s0:s0+512] = rope(ps * scale), tables at cols s0."""
                dst = dst_tile[:, s0:s0 + 512]
                c_sl = tab_ck[:, s0:s0 + 512]
                s_sl = tab_sk[:, s0:s0 + 512]
                sw = swpool.tile([P, 512], F32, tag="sw")
                for o in range(0, P, 64):
                    nc.vector.scalar_tensor_tensor(
                        sw[o:o + 32, :], ps[o + 32:o + 64, :], scale,
                        s_sl[o:o + 32, :], MULT, MULT)
                    nc.vector.scalar_tensor_tensor(
                        sw[o + 32:o + 64, :], ps[o:o + 32, :], scale,
                        s_sl[o + 32:o + 64, :], MULT, MULT)
                nc.vector.scalar_tensor_tensor(
                    dst, ps[:], scale, c_sl, MULT, MULT)
                nc.vector.tensor_tensor(dst, dst, sw[:], ADD)

            def make_xq(xpool, s0, name=""):
                out = []
                for kq in range(NKQ):
                    xq = xpool.tile([P, KH, 512], F32R, tag="xq",
                                    name=f"xq{name}{kq}")
                    nc.sync.dma_start(
                        xq[:], xT_r[:, kq * KH:(kq + 1) * KH, s0:s0 + 512])
                    out.append(xq)
                return out

            def proj_mm(ps, xqs, w_sb, mlo, mhi, rot=0):
                # rotated contraction order staggers x quarter-tile frees
                ks = [(rot * KH + i) % KO for i in range(KO)]
                for i, k in enumerate(ks):
                    nc.tensor.matmul(
                        ps[:], w_sb[:, k, mlo:mhi],
                        xqs[k // KH][:, k % KH, :],
                        start=(i == 0), stop=(i == KO - 1))

            attn = qsb

            def attention_qc(qc, fill, nf):
                """One 512-query chunk of attention for all 4 head pairs,
                popping `nf` filler steps per kc iteration."""
                for p in range(4):
                    pvA = pvpool.tile([P, 512], F32, tag="pv",
                                      name=f"pvA_{rep}")
                    pvB = pvpool.tile([P, 512], F32, tag="pv",
                                      name=f"pvB_{rep}")
                    pvs = (pvA, pvB)
                    for kc in range(KO):
                        ss = spool.tile([P, 1024], F32, tag="ss")
                        for i in range(2):
                            nc.tensor.matmul(
                                ss[:, i * 512:(i + 1) * 512],
                                kab[i * HD:(i + 1) * HD,
                                    kc * P:(kc + 1) * P],
                                qsb[p][i * HD:(i + 1) * HD,
                                       qc * 512:(qc + 1) * 512],
                                start=True, stop=True,
                                tile_position=(i * HD, 0))
                        pr = prpool.tile([P, 1024], F32R, tag="pr")
                        nc.scalar.activation(pr[:], ss[:], EXP)
                        for i in range(2):
                            nc.tensor.matmul(
                                pvs[i][0:HD + 1, :],
                                vpr[i][:, kc, :],
                                pr[:, i * 512:(i + 1) * 512],
                                start=(kc == 0), stop=(kc == KO - 1))
                        for _ in range(nf):
                            step = next(fill, None)
                            if step is not None and step is not PAD:
                                step()
                    for i in range(2):
                        qb = i * HD
                        # stage PV to SBUF in one copy so the psum bank
                        # frees for the next pair immediately; DVE operand
                        # partition bases must be 32-aligned, and custom-DVE
                        # ops need base 0, hence z at row HD + zrow copy.
                        pvs_sb = nrmpool.tile([HD + 1, 512], F32,
                                              tag="pvsb")
                        nc.vector.tensor_copy(pvs_sb[:], pvs[i][0:HD + 1, :])
                        zrow = nrmpool.tile([1, 512], F32, tag="zrow")
                        nc.vector.tensor_copy(zrow[:], pvs_sb[HD:HD + 1, :])
                        rz = nrmpool.tile([1, 512], F32, tag="rz")
                        nc.vector.reciprocal_approx_fast(rz[:], zrow[:])
                        rzb = nrmpool.tile([HD, 512], F32, tag="rzb")
                        nc.gpsimd.partition_broadcast(rzb[:], rz[:])
                        nc.vector.tensor_tensor(
                            attn[p][qb:qb + HD, qc * 512:(qc + 1) * 512],
                            pvs_sb[0:HD, :], rzb[:], MULT)
                # drain leftover fillers for this phase
                for step in fill:
                    if step is not PAD:
                        step()

            PAD = object()

            def chain(*gens):
                for g in gens:
                    yield from g

            def interleave(a, b):
                a, b = iter(a), iter(b)
                while True:
                    got = False
                    for it in (a, b):
                        try:
                            yield next(it)
                            got = True
                        except StopIteration:
                            pass
                    if not got:
                        return

            def pad(n, gen):
                for _ in range(n):
                    yield PAD
                yield from gen

            # ------- phase 1: preamble (K/V/V'/Q-chunk0) + attention qc0,qc1
            # (q-projection fillers; wq + x pools live through this phase)
            with tc.tile_pool(name="projX", bufs=14) as xpool, \
                 tc.tile_pool(name="wqp", bufs=1) as wqpool, \
                 tc.tile_pool(name="projKVW", bufs=1) as kvwpool:
                wk_sb = kvwpool.tile([P, KO, NKV], F32R, tag="wk")
                nc.sync.dma_start(wk_sb[:], wkT_r)
                xqs0 = make_xq(xpool, 0, "p0")
                wv_sb = kvwpool.tile([P, KO, NKV], F32R, tag="wv")
                nc.sync.dma_start(wv_sb[:], wvT_r)
                nc.sync.dma_start(tab_ck[:], ck.ap())
                nc.sync.dma_start(tab_sk[:], sk.ap())
                wq_sb = wqpool.tile([P, KO, NQ], F32R, tag="wq")
                nc.sync.dma_start(wq_sb[:], wqT_r)

                def kv_proj(n4, xqs):
                    s0 = n4 * 512
                    ps = mmps.tile([P, 512], F32, tag="mm")
                    proj_mm(ps, xqs, wk_sb, 0, NKV, rot=0)
                    rope_evac(ps, kab, s0, 1.0)
                    ps = mmps.tile([P, 512], F32, tag="mm")
                    proj_mm(ps, xqs, wv_sb, 0, NKV, rot=1)
                    vsb = swpool.tile([P, 512], F32, tag="vsb")
                    nc.scalar.activation(vsb[:], ps[:], COPY)
                    for i in range(2):
                        for kq in range(4):
                            kc = n4 * 4 + kq
                            pst = mmps.tile([P, 512], F32, tag="mm",
                                            name="pst")[:, 0:HD]
                            nc.tensor.transpose(
                                pst[:],
                                vsb[i * HD:(i + 1) * HD,
                                    kq * P:(kq + 1) * P],
                                ident[i * HD:(i + 1) * HD,
                                      i * HD:(i + 1) * HD])
                            nc.scalar.activation(
                                vpr[i][:, kc, 0:HD], pst[:], COPY,
                                scale=msk[:, kc:kc + 1])

                for n4 in range(SC):
                    xqs = xqs0 if n4 == 0 else make_xq(xpool, n4 * 512,
                                                       f"p{n4}")
                    kv_proj(n4, xqs)
                    # Q chunk 0 tile m rides each n4 slot: PE work fills
                    # the x-DMA gaps, chunk-0 queries ready at attention
                    # start.
                    ps = mmps.tile([P, 512], F32, tag="mm")
                    proj_mm(ps, xqs0, wq_sb, n4 * P, (n4 + 1) * P,
                            rot=(2 + n4) % NKQ)
                    rope_evac(ps, qsb[n4], 0, 0.125)
                for i in range(2):
                    nc.vector.tensor_copy(vpr[i][:, :, HD], msk[:])

                def q_steps(n4):
                    """Single-matmul steps projecting q chunk n4 (4 m)."""
                    box = {}
                    for m in range(4):
                        rot = (2 + m) % NKQ
                        ks = [(rot * KH + i) % KO for i in range(KO)]
                        for i, k in enumerate(ks):
                            def f(n4=n4, m=m, i=i, k=k):
                                if m == 0 and i == 0:
                                    box["xq"] = make_xq(xpool, n4 * 512,
                                                        f"a{n4}")
                                if i == 0:
                                    box["ps"] = mmps.tile(
                                        [P, 512], F32, tag="mm",
                                        name=f"qa{n4}_{m}_{rep}")
                                nc.tensor.matmul(
                                    box["ps"][:],
                                    wq_sb[:, k, m * P:(m + 1) * P],
                                    box["xq"][k // KH][:, k % KH, :],
                                    start=(i == 0), stop=(i == KO - 1))
                                if i == KO - 1:
                                    rope_evac(box["ps"], qsb[m], n4 * 512,
                                              0.125)
                            yield f

                attention_qc(0, pad(10, q_steps(1)), 1)
                attention_qc(1, pad(6, chain(q_steps(2), q_steps(3))), 2)

            # ------- phase 2: attention qc2,qc3 (wo fillers) + wo tail
            with tc.tile_pool(name="wo", bufs=1) as wopool, \
                 tc.tile_pool(name="oev", bufs=4) as oevpool:
                wot_sb = wopool.tile([P, 4, S], F32R, tag="wot_sb")
                nc.sync.dma_start(wot_sb[:], woT_r)

                def wo_steps(qt, on_act=False):
                    """Single-matmul steps for wo chunk qt (4 n-chunks)."""
                    box = {}
                    for n in range(SC):
                        for dk in range(4):
                            def f(qt=qt, n=n, dk=dk, on_act=on_act):
                                if dk == 0:
                                    box["po"] = mmps.tile(
                                        [P, 512], F32, tag="mm",
                                        name=f"wo{qt}_{n}_{rep}")
                                nc.tensor.matmul(
                                    box["po"][:],
                                    attn[dk][:, qt * P:(qt + 1) * P],
                                    wot_sb[:, dk, n * 512:(n + 1) * 512],
                                    start=(dk == 0), stop=(dk == 3))
                                if dk == 3:
                                    ot = oevpool.tile([P, 512], F32,
                                                      tag="ot")
                                    if on_act:
                                        nc.scalar.activation(
                                            ot[:], box["po"][:], COPY)
                                    else:
                                        nc.vector.tensor_copy(
                                            ot[:], box["po"][:])
                                    nc.sync.dma_start(
                                        part.ap()[qt * P:(qt + 1) * P,
                                                  n * 512:(n + 1) * 512],
                                        ot[:])
                            yield f

                attention_qc(2, pad(12, interleave(
                    chain(wo_steps(0), wo_steps(1), wo_steps(2),
                          wo_steps(3)),
                    chain(wo_steps(4), wo_steps(5), wo_steps(6),
                          wo_steps(7)))), 2)
                attention_qc(3, chain(wo_steps(8), wo_steps(9),
                                      wo_steps(10), wo_steps(11)), 1)

                # tail: wo chunks 12-15 (ACT idle; evac alternates DVE/ACT)
                for qt in range(12, KO):
                    for step in wo_steps(qt, on_act=(qt % 2 == 0)):
                        step()

    nc.compile()
    return nc


_PERM = np.concatenate([np.arange(0, HD, 2), np.arange(1, HD, 2)])


def _round_fp32r(a):
    """Round float32 to fp32r (low 12 mantissa bits dropped, nearest-even)."""
    b = np.ascontiguousarray(a, dtype=np.float32).view(np.uint32)
    lsb = (b >> 12) & 1
    out = ((b + 0x7FF + lsb) & 0xFFFFF000).astype(np.uint32)
    return out.view(np.float32)


def _prep_core_inputs(x, wq, wk, wv, wo, attention_mask, core, tables):
    b = core // 4
    g = core % 4
    ctab, stab = tables

    # head order [0,4,1,5,2,6,3,7]: tile m holds heads (m, m+4) so head h
    # sits at partition base (h//4)*64 == its kv head's base in kab
    hperm = np.array([0, 4, 1, 5, 2, 6, 3, 7])
    qrows = wq[8 * g * HD:(8 * g + 8) * HD]          # [512, 2048]
    qrows = qrows.reshape(8, HD, D)[hperm][:, _PERM, :].reshape(NQ, D)
    krows = wk[2 * g * HD:(2 * g + 2) * HD]          # [128, 2048]
    krows = krows.reshape(2, HD, D)[:, _PERM, :].reshape(NKV, D)
    vrows = wv[2 * g * HD:(2 * g + 2) * HD]          # [128, 2048]
    wocols = wo[:, 8 * g * HD:(8 * g + 8) * HD]      # [2048, 512]
    wocols = wocols.reshape(D, 8, HD)[:, hperm, :].reshape(D, NQ)

    maskf = attention_mask[b].astype(np.float32)     # [S]
    maskT = np.ascontiguousarray(maskf.reshape(KO, P).T)   # [128, 16]

    return {
        "xT": _round_fp32r(x[b].T),
        "wqT": _round_fp32r(qrows.T),
        "wkT": _round_fp32r(krows.T),
        "wvT": _round_fp32r(vrows.T),
        "woT": _round_fp32r(wocols.T),
        "ck": ctab,
        "sk": stab,
        "maskT": maskT,
    }


_CACHED_NC = None


def _get_nc():
    global _CACHED_NC
    if _CACHED_NC is None:
        _CACHED_NC = _build_bass()
    return _CACHED_NC


def _make_in_maps(x, wq, wk, wv, wo, attention_mask):
    tables = _rope_tables()
    return [
        _prep_core_inputs(x, wq, wk, wv, wo, attention_mask, c, tables)
        for c in range(N_CORES)
    ]


def kernel(x, wq, wk, wv, wo, attention_mask):
    from concourse.bass_utils import run_bass_kernel_spmd

    x = np.asarray(x, dtype=np.float32)
    wq = np.asarray(wq, dtype=np.float32)
    wk = np.asarray(wk, dtype=np.float32)
    wv = np.asarray(wv, dtype=np.float32)
    wo = np.asarray(wo, dtype=np.float32)
    attention_mask = np.asarray(attention_mask)

    nc = _get_nc()
    in_maps = _make_in_maps(x, wq, wk, wv, wo, attention_mask)
    res = run_bass_kernel_spmd(nc, in_maps, core_ids=list(range(N_CORES)))
    out = np.zeros((B, S, D), dtype=np.float32)
    for c in range(N_CORES):
        out[c // 4] += res.results[c]["part"]
    return out


if __name__ == "__main__":
    rng = np.random.default_rng(0)
    ins = {
        "x": rng.standard_normal((B, S, D), dtype=np.float32),
        "wq": (rng.standard_normal((H * HD, D)) * 0.02).astype(np.float32),
        "wk": (rng.standard_normal((KVH * HD, D)) * 0.02).astype(np.float32),
        "wv": (rng.standard_normal((KVH * HD, D)) * 0.02).astype(np.float32),
        "wo": (rng.standard_normal((D, H * HD)) * 0.02).astype(np.float32),
        "attention_mask": np.ones((B, S), dtype=np.int32),
    }
    out = kernel(**ins)
    print("kernel ran, out shape", out.shape, "std", out.std())


# revision 14
# speedup vs baseline: 1.2271x; 1.2271x over previous
"""GQA attention block (B=2, S=2048, D=2048, H=32, KVH=8, HD=64, RoPE) on 8
Trainium2 NeuronCores.

Sharding: core = (batch, kv-head pair). Core c handles batch c//4 and kv heads
{2*(c%4), 2*(c%4)+1} (i.e. q heads 8*(c%4)..8*(c%4)+7). Each core runs the full
chain for its heads: q/k/v projections + RoPE, attention, and its row-slice of
the output projection; the host sums the 4 partial wo-outputs per batch.

Device-side layout (host-side transforms are free):
- x is passed transposed (xT [D, S]) so projections produce qT/kT/vT with the
  head dim on partitions.
- RoPE uses the "half layout": wq/wk rows permuted per head to
  [even dims, odd dims]; cos/sin tables precomputed host-side. The 1/sqrt(HD)
  score scale is folded into the Q tables' consumer (scale=0.125).
- Scores are computed transposed ([keys, queries]) so softmax exp is a pure
  elementwise ACT op and the PV matmul consumes probsT directly as the moving
  operand. exp skips max-subtraction (scores bounded ~|7|).
- The softmax normalizer z is row HD of the PV output: V' carries a mask
  column at index HD (ones * mask), so PV yields z for free.
- All matmuls float32r (full-rate fp32 streaming).

Schedule: the attention phase is ACT(exp)-bound (~1.04us per kc iteration vs
~0.65us of PE work). All remaining PE work (q projections for chunks 1-3, the
first 12 wo chunks) is sliced into single-matmul "filler" steps issued between
kc iterations so the exp stream never starves. Preamble (K/V/V'/Q-chunk0) is
x-DMA-bound; attention starts as soon as K is fully projected.
"""

import sys

import numpy as np

if "/opt/trn_rl_repo" not in sys.path:
    sys.path.insert(0, "/opt/trn_rl_repo")

B, S, D = 2, 2048, 2048
H, KVH = 32, 8
HD = D // H            # 64
NREP = H // KVH        # 4
ROPE_THETA = 10000.0
N_CORES = 8
P = 128
NQ = 512               # q rows per core (8 heads * 64)
NKV = 128              # k/v rows per core (2 kv heads * 64)
KO = D // P            # 16 contraction chunks for projections
SC = S // 512          # 4 column chunks of 512
KH = 2                 # x streams in [P, KH, 512] tiles
NKQ = KO // KH         # 8 quarter-tiles per column chunk


def _rope_tables():
    """cos/sin tables [P, S] matching the qT/kT partition layout.

    Partition layout per 64-row head block: rows 0:32 = "a" (even dims),
    rows 32:64 = "b" (odd dims). a' = a*cos - b*sin ; b' = a*sin + b*cos.
    The in0 of the fused swap-multiply reads the OTHER block, so the sin
    table carries -sin on a-rows and +sin on b-rows.
    """
    freqs = (1.0 / (ROPE_THETA **
                    (np.arange(0, HD, 2, dtype=np.float32) / np.float32(HD))))
    freqs = freqs.astype(np.float32)                                  # [32]
    ang = (np.arange(S, dtype=np.float32)[None, :] * freqs[:, None])  # [32, S]
    cos = np.cos(ang).astype(np.float32)
    sin = np.sin(ang).astype(np.float32)
    ctab = np.concatenate([cos, cos, cos, cos], axis=0)               # [128, S]
    stab = np.concatenate([-sin, sin, -sin, sin], axis=0)             # [128, S]
    return ctab, stab


def _build_bass(reps: int = 1):
    import concourse.bass as bass  # noqa: F401
    import concourse.tile as tile
    from concourse import bacc, mybir
    from concourse.masks import make_identity

    F32 = mybir.dt.float32
    F32R = mybir.dt.float32r
    EXP = mybir.ActivationFunctionType.Exp
    COPY = mybir.ActivationFunctionType.Copy
    MULT = mybir.AluOpType.mult
    ADD = mybir.AluOpType.add

    nc = bacc.Bacc("TRN2", target_bir_lowering=False, debug=False,
                   num_devices=N_CORES)

    xT = nc.dram_tensor("xT", [D, S], F32R, kind="ExternalInput")
    wqT = nc.dram_tensor("wqT", [D, NQ], F32R, kind="ExternalInput")
    wkT = nc.dram_tensor("wkT", [D, NKV], F32R, kind="ExternalInput")
    wvT = nc.dram_tensor("wvT", [D, NKV], F32R, kind="ExternalInput")
    woT = nc.dram_tensor("woT", [NQ, D], F32R, kind="ExternalInput")
    ck = nc.dram_tensor("ck", [P, S], F32, kind="ExternalInput")
    sk = nc.dram_tensor("sk", [P, S], F32, kind="ExternalInput")
    maskT = nc.dram_tensor("maskT", [P, KO], F32, kind="ExternalInput")
    part = nc.dram_tensor("part", [S, D], F32, kind="ExternalOutput")

    xT_r = xT.ap().rearrange("(ko p) s -> p ko s", p=P)     # [128, 16, 2048]
    wqT_r = wqT.ap().rearrange("(ko p) m -> p ko m", p=P)   # [128, 16, 512]
    wkT_r = wkT.ap().rearrange("(ko p) m -> p ko m", p=P)   # [128, 16, 128]
    wvT_r = wvT.ap().rearrange("(ko p) m -> p ko m", p=P)   # [128, 16, 128]
    woT_r = woT.ap().rearrange("(dk p) e -> p dk e", p=P)   # [128, 4, 2048]

    with tile.TileContext(nc) as tc:
      for rep in range(reps):
        with tc.tile_pool(name="persist", bufs=1) as persist, \
             tc.tile_pool(name="probs", bufs=3) as prpool, \
             tc.tile_pool(name="nrm", bufs=2) as nrmpool, \
             tc.tile_pool(name="sw", bufs=2) as swpool, \
             tc.tile_pool(name="mmPS", bufs=2, space="PSUM") as mmps, \
             tc.tile_pool(name="attnPS", bufs=2, space="PSUM") as spool, \
             tc.tile_pool(name="pvPS", bufs=2, space="PSUM") as pvpool:

            qsb = [persist.tile([P, S], F32R, tag=f"qsb{m}", name=f"qsb{m}_{rep}")
                   for m in range(4)]
            kab = persist.tile([P, S], F32R, tag="kab")
            # V' layout: col 0 = mask (softmax z source), cols 1:65 = V dims
            vpr = [persist.tile([P, KO, HD + 1], F32R, tag=f"vpr{i}",
                                name=f"vpr{i}_{rep}")
                   for i in range(2)]
            msk = persist.tile([P, KO], F32, tag="msk")
            ident = persist.tile([P, P], F32, tag="ident")
            tab_ck = persist.tile([P, S], F32, tag="tab_ck")
            tab_sk = persist.tile([P, S], F32, tag="tab_sk")

            nc.sync.dma_start(msk[:], maskT.ap())
            make_identity(nc, ident[:])

            def rope_evac(ps, dst_tile, s0, scale):
                """dst[:, s0:s0+512] = rope(ps * scale), tables at cols s0."""
                dst = dst_tile[:, s0:s0 + 512]
                c_sl = tab_ck[:, s0:s0 + 512]
                s_sl = tab_sk[:, s0:s0 + 512]
                sw = swpool.tile([P, 512], F32, tag="sw")
                for o in range(0, P, 64):
                    nc.vector.scalar_tensor_tensor(
                        sw[o:o + 32, :], ps[o + 32:o + 64, :], scale,
                        s_sl[o:o + 32, :], MULT, MULT)
                    nc.vector.scalar_tensor_tensor(
                        sw[o + 32:o + 64, :], ps[o:o + 32, :], scale,
                        s_sl[o + 32:o + 64, :], MULT, MULT)
                nc.vector.scalar_tensor_tensor(
                    dst, ps[:], scale, c_sl, MULT, MULT)
                nc.vector.tensor_tensor(dst, dst, sw[:], ADD)

            def make_xq(xpool, s0, name=""):
                out = []
                for kq in range(NKQ):
                    xq = xpool.tile([P, KH, 512], F32R, tag="xq",
                                    name=f"xq{name}{kq}")
                    nc.sync.dma_start(
                        xq[:], xT_r[:, kq * KH:(kq + 1) * KH, s0:s0 + 512])
                    out.append(xq)
                return out

            def proj_mm(ps, xqs, w_sb, mlo, mhi, rot=0):
                # rotated contraction order staggers x quarter-tile frees
                ks = [(rot * KH + i) % KO for i in range(KO)]
                for i, k in enumerate(ks):
                    nc.tensor.matmul(
                        ps[:], w_sb[:, k, mlo:mhi],
                        xqs[k // KH][:, k % KH, :],
                        start=(i == 0), stop=(i == KO - 1))

            attn = qsb

            def attention_qc(qc, fill, nf, dense=False):
                """One 512-query chunk of attention for all 4 head pairs,
                popping `nf` filler steps per kc iteration."""
                dmy = mmps.tile([P, 512], F32, tag="mm",
                                name=f"dmy{qc}_{rep}") if dense else None
                for p in range(4):
                    pvA = pvpool.tile([P, 512], F32, tag="pv",
                                      name=f"pvA_{rep}")
                    pvB = pvpool.tile([P, 512], F32, tag="pv",
                                      name=f"pvB_{rep}")
                    pvs = (pvA, pvB)
                    for kc in range(KO):
                        ss = spool.tile([P, 1024], F32, tag="ss")
                        for i in range(2):
                            nc.tensor.matmul(
                                ss[:, i * 512:(i + 1) * 512],
                                kab[i * HD:(i + 1) * HD,
                                    kc * P:(kc + 1) * P],
                                qsb[p][i * HD:(i + 1) * HD,
                                       qc * 512:(qc + 1) * 512],
                                start=True, stop=True,
                                tile_position=(i * HD, 0))
                        pr = prpool.tile([P, 1024], F32R, tag="pr")
                        nc.scalar.activation(pr[:], ss[:], EXP)
                        for i in range(2):
                            nc.tensor.matmul(
                                pvs[i][0:HD + 1, :],
                                vpr[i][:, kc, :],
                                pr[:, i * 512:(i + 1) * 512],
                                start=(kc == 0), stop=(kc == KO - 1))
                        got = 0
                        for _ in range(nf):
                            step = next(fill, None)
                            if step is not None and step is not PAD:
                                step()
                                got += 1
                        if got == 0 and dmy is not None:
                            # dummy matmul: keeps the PE activity monitor
                            # busy so the clock gate stays at full rate
                            nc.tensor.matmul(
                                dmy[:], kab[0:HD, 0:P],
                                qsb[0][0:HD, 0:512],
                                start=True, stop=True)
                    for i in range(2):
                        qb = i * HD
                        # stage PV to SBUF in one copy so the psum bank
                        # frees for the next pair immediately; DVE operand
                        # partition bases must be 32-aligned, and custom-DVE
                        # ops need base 0, hence z at row HD + zrow copy.
                        pvs_sb = nrmpool.tile([HD + 1, 512], F32,
                                              tag="pvsb")
                        nc.vector.tensor_copy(pvs_sb[:], pvs[i][0:HD + 1, :])
                        zrow = nrmpool.tile([1, 512], F32, tag="zrow")
                        nc.vector.tensor_copy(zrow[:], pvs_sb[HD:HD + 1, :])
                        rz = nrmpool.tile([1, 512], F32, tag="rz")
                        nc.vector.reciprocal_approx_fast(rz[:], zrow[:])
                        rzb = nrmpool.tile([HD, 512], F32, tag="rzb")
                        nc.gpsimd.partition_broadcast(rzb[:], rz[:])
                        nc.vector.tensor_tensor(
                            attn[p][qb:qb + HD, qc * 512:(qc + 1) * 512],
                            pvs_sb[0:HD, :], rzb[:], MULT)
                # drain leftover fillers for this phase
                for step in fill:
                    if step is not PAD:
                        step()

            PAD = object()

            def chain(*gens):
                for g in gens:
                    yield from g

            def interleave(a, b):
                a, b = iter(a), iter(b)
                while True:
                    got = False
                    for it in (a, b):
                        try:
                            yield next(it)
                            got = True
                        except StopIteration:
                            pass
                    if not got:
                        return

            def pad(n, gen):
                for _ in range(n):
                    yield PAD
                yield from gen

            # ------- phase 1: preamble (K/V/V'/Q-chunk0) + attention qc0,qc1
            # (q-projection fillers; wq + x pools live through this phase)
            with tc.tile_pool(name="projX", bufs=14) as xpool, \
                 tc.tile_pool(name="wqp", bufs=1) as wqpool, \
                 tc.tile_pool(name="projKVW", bufs=1) as kvwpool:
                wk_sb = kvwpool.tile([P, KO, NKV], F32R, tag="wk")
                nc.sync.dma_start(wk_sb[:], wkT_r)
                xqs0 = make_xq(xpool, 0, "p0")
                wv_sb = kvwpool.tile([P, KO, NKV], F32R, tag="wv")
                nc.sync.dma_start(wv_sb[:], wvT_r)
                nc.sync.dma_start(tab_ck[:], ck.ap())
                nc.sync.dma_start(tab_sk[:], sk.ap())
                wq_sb = wqpool.tile([P, KO, NQ], F32R, tag="wq")
                nc.sync.dma_start(wq_sb[:], wqT_r)

                def kv_proj(n4, xqs):
                    s0 = n4 * 512
                    ps = mmps.tile([P, 512], F32, tag="mm")
                    proj_mm(ps, xqs, wk_sb, 0, NKV, rot=0)
                    rope_evac(ps, kab, s0, 1.0)
                    ps = mmps.tile([P, 512], F32, tag="mm")
                    proj_mm(ps, xqs, wv_sb, 0, NKV, rot=1)
                    vsb = swpool.tile([P, 512], F32, tag="vsb")
                    nc.scalar.activation(vsb[:], ps[:], COPY)
                    for i in range(2):
                        for kq in range(4):
                            kc = n4 * 4 + kq
                            pst = mmps.tile([P, 512], F32, tag="mm",
                                            name="pst")[:, 0:HD]
                            nc.tensor.transpose(
                                pst[:],
                                vsb[i * HD:(i + 1) * HD,
                                    kq * P:(kq + 1) * P],
                                ident[i * HD:(i + 1) * HD,
                                      i * HD:(i + 1) * HD])
                            nc.scalar.activation(
                                vpr[i][:, kc, 0:HD], pst[:], COPY,
                                scale=msk[:, kc:kc + 1])

                for n4 in range(SC):
                    xqs = xqs0 if n4 == 0 else make_xq(xpool, n4 * 512,
                                                       f"p{n4}")
                    kv_proj(n4, xqs)
                    # Q chunk 0 tile m rides each n4 slot: PE work fills
                    # the x-DMA gaps, chunk-0 queries ready at attention
                    # start.
                    ps = mmps.tile([P, 512], F32, tag="mm")
                    proj_mm(ps, xqs0, wq_sb, n4 * P, (n4 + 1) * P,
                            rot=(2 + n4) % NKQ)
                    rope_evac(ps, qsb[n4], 0, 0.125)
                for i in range(2):
                    nc.vector.tensor_copy(vpr[i][:, :, HD], msk[:])

                def q_steps(n4):
                    """Single-matmul steps projecting q chunk n4 (4 m)."""
                    box = {}
                    for m in range(4):
                        rot = (2 + m) % NKQ
                        ks = [(rot * KH + i) % KO for i in range(KO)]
                        for i, k in enumerate(ks):
                            def f(n4=n4, m=m, i=i, k=k):
                                if m == 0 and i == 0:
                                    box["xq"] = make_xq(xpool, n4 * 512,
                                                        f"a{n4}")
                                if i == 0:
                                    box["ps"] = mmps.tile(
                                        [P, 512], F32, tag="mm",
                                        name=f"qa{n4}_{m}_{rep}")
                                nc.tensor.matmul(
                                    box["ps"][:],
                                    wq_sb[:, k, m * P:(m + 1) * P],
                                    box["xq"][k // KH][:, k % KH, :],
                                    start=(i == 0), stop=(i == KO - 1))
                                if i == KO - 1:
                                    rope_evac(box["ps"], qsb[m], n4 * 512,
                                              0.125)
                            yield f

                attention_qc(0, pad(10, chain(q_steps(1), q_steps(2))), 2)
                attention_qc(1, pad(4, q_steps(3)), 1, dense=True)

            # ------- phase 2: attention qc2,qc3 (wo fillers) + wo tail
            with tc.tile_pool(name="wo", bufs=1) as wopool, \
                 tc.tile_pool(name="oev", bufs=4) as oevpool:
                wot_sb = wopool.tile([P, 4, S], F32R, tag="wot_sb")
                nc.sync.dma_start(wot_sb[:], woT_r)

                def wo_steps(qt, on_act=False):
                    """Single-matmul steps for wo chunk qt (4 n-chunks)."""
                    box = {}
                    for n in range(SC):
                        for dk in range(4):
                            def f(qt=qt, n=n, dk=dk, on_act=on_act):
                                if dk == 0:
                                    box["po"] = mmps.tile(
                                        [P, 512], F32, tag="mm",
                                        name=f"wo{qt}_{n}_{rep}")
                                nc.tensor.matmul(
                                    box["po"][:],
                                    attn[dk][:, qt * P:(qt + 1) * P],
                                    wot_sb[:, dk, n * 512:(n + 1) * 512],
                                    start=(dk == 0), stop=(dk == 3))
                                if dk == 3:
                                    ot = oevpool.tile([P, 512], F32,
                                                      tag="ot")
                                    if on_act:
                                        nc.scalar.activation(
                                            ot[:], box["po"][:], COPY)
                                    else:
                                        nc.vector.tensor_copy(
                                            ot[:], box["po"][:])
                                    nc.sync.dma_start(
                                        part.ap()[qt * P:(qt + 1) * P,
                                                  n * 512:(n + 1) * 512],
                                        ot[:])
                            yield f

                attention_qc(2, pad(12, interleave(
                    chain(wo_steps(0), wo_steps(1), wo_steps(2),
                          wo_steps(3)),
                    chain(wo_steps(4), wo_steps(5), wo_steps(6),
                          wo_steps(7)))), 2)
                attention_qc(3, chain(wo_steps(8), wo_steps(9),
                                      wo_steps(10), wo_steps(11)), 1,
                             dense=True)

                # tail: wo chunks 12-15 (ACT idle; evac alternates DVE/ACT)
                for qt in range(12, KO):
                    for step in wo_steps(qt, on_act=(qt % 2 == 0)):
                        step()

    nc.compile()
    return nc


_PERM = np.concatenate([np.arange(0, HD, 2), np.arange(1, HD, 2)])


def _round_fp32r(a):
    """Round float32 to fp32r (low 12 mantissa bits dropped, nearest-even)."""
    b = np.ascontiguousarray(a, dtype=np.float32).view(np.uint32)
    lsb = (b >> 12) & 1
    out = ((b + 0x7FF + lsb) & 0xFFFFF000).astype(np.uint32)
    return out.view(np.float32)


def _prep_core_inputs(x, wq, wk, wv, wo, attention_mask, core, tables):
    b = core // 4
    g = core % 4
    ctab, stab = tables

    # head order [0,4,1,5,2,6,3,7]: tile m holds heads (m, m+4) so head h
    # sits at partition base (h//4)*64 == its kv head's base in kab
    hperm = np.array([0, 4, 1, 5, 2, 6, 3, 7])
    qrows = wq[8 * g * HD:(8 * g + 8) * HD]          # [512, 2048]
    qrows = qrows.reshape(8, HD, D)[hperm][:, _PERM, :].reshape(NQ, D)
    krows = wk[2 * g * HD:(2 * g + 2) * HD]          # [128, 2048]
    krows = krows.reshape(2, HD, D)[:, _PERM, :].reshape(NKV, D)
    vrows = wv[2 * g * HD:(2 * g + 2) * HD]          # [128, 2048]
    wocols = wo[:, 8 * g * HD:(8 * g + 8) * HD]      # [2048, 512]
    wocols = wocols.reshape(D, 8, HD)[:, hperm, :].reshape(D, NQ)

    maskf = attention_mask[b].astype(np.float32)     # [S]
    maskT = np.ascontiguousarray(maskf.reshape(KO, P).T)   # [128, 16]

    return {
        "xT": _round_fp32r(x[b].T),
        "wqT": _round_fp32r(qrows.T),
        "wkT": _round_fp32r(krows.T),
        "wvT": _round_fp32r(vrows.T),
        "woT": _round_fp32r(wocols.T),
        "ck": ctab,
        "sk": stab,
        "maskT": maskT,
    }


_CACHED_NC = None


def _get_nc():
    global _CACHED_NC
    if _CACHED_NC is None:
        _CACHED_NC = _build_bass()
    return _CACHED_NC


def _make_in_maps(x, wq, wk, wv, wo, attention_mask):
    tables = _rope_tables()
    return [
        _prep_core_inputs(x, wq, wk, wv, wo, attention_mask, c, tables)
        for c in range(N_CORES)
    ]


def kernel(x, wq, wk, wv, wo, attention_mask):
    from concourse.bass_utils import run_bass_kernel_spmd

    x = np.asarray(x, dtype=np.float32)
    wq = np.asarray(wq, dtype=np.float32)
    wk = np.asarray(wk, dtype=np.float32)
    wv = np.asarray(wv, dtype=np.float32)
    wo = np.asarray(wo, dtype=np.float32)
    attention_mask = np.asarray(attention_mask)

    nc = _get_nc()
    in_maps = _make_in_maps(x, wq, wk, wv, wo, attention_mask)
    res = run_bass_kernel_spmd(nc, in_maps, core_ids=list(range(N_CORES)))
    out = np.zeros((B, S, D), dtype=np.float32)
    for c in range(N_CORES):
        out[c // 4] += res.results[c]["part"]
    return out


if __name__ == "__main__":
    rng = np.random.default_rng(0)
    ins = {
        "x": rng.standard_normal((B, S, D), dtype=np.float32),
        "wq": (rng.standard_normal((H * HD, D)) * 0.02).astype(np.float32),
        "wk": (rng.standard_normal((KVH * HD, D)) * 0.02).astype(np.float32),
        "wv": (rng.standard_normal((KVH * HD, D)) * 0.02).astype(np.float32),
        "wo": (rng.standard_normal((D, H * HD)) * 0.02).astype(np.float32),
        "attention_mask": np.ones((B, S), dtype=np.int32),
    }
    out = kernel(**ins)
    print("kernel ran, out shape", out.shape, "std", out.std())
